# revision 3
# baseline (speedup 1.0000x reference)
"""Binarized 3x3 conv (N=32, C=256->256, H=W=56, pad 1) on 8 TRN2 NeuronCores.

Sharding: data-parallel over batch (4 images per core), weights replicated.

Math: binarize exactly via
  xb = (x >= 0) - 0.5            in {+-0.5}  (exact in fp8 e4m3)
  wb = (w >= 0) - 0.5            in {+-0.5}  (exact in fp8 e4m3)
so every product is exactly +-0.25 and fp32 PSUM accumulation is exact
(quarter-integer partial sums, |.| <= 576 << 2^22). The output drain applies
scale=4.0 to restore the +-1-product conv result. sign(0)=+1 is honored.

Conv as matmul: the padded (58x58) binarized image lives flat in SBUF, so for
each kernel tap (kh,kw) the needed input window is a CONTIGUOUS span of the
flat padded grid shifted by (kh-1)*58+(kw-1). Outputs are computed on the
padded grid (464-wide spans = 8 padded rows) and the two garbage columns per
row (conv centered on pad columns) are dropped at drain time.

TensorE: fp8 DoubleRow matmuls contract all 256 input channels in one
instruction (K=128 partitions x 2 interleaved weights/cell), 9 accumulating
matmuls (one per tap) per output tile. 2 co-chunks x 4 images x 7 row-groups
x 9 taps = 504 matmuls per core.

Weights: ONE contiguous DMA loads w[o, i, kh, kw] as [o_local=128 part,
(oc, i, tap)] (256 descriptors of 9216B — the HBM-contiguous axis (i, tap)
lands on the SBUF free axis). The o<->i transpose needed for the matmul
lhsT layout [ci_local][two][co] is done on-chip: 36 PE transpose-mode
matmuls of 128x128 f32 blocks (strided columns, stride 9) into PSUM, each
drained by a DVE tensor_scalar that fuses the binarize to {+-0.5} fp8 and
scatters into the DoubleRow layout [tap][two][co]. This replaces the old
36B-run gather DMA (131072 descriptors, ~38 ms) with ~25 us of work.
"""

import os
os.environ.setdefault("CONCOURSE_SCRUB_NEFF_DEBUG_INFO", "1")

import numpy as np

import concourse.bass as bass
import concourse.mybir as mybir
import concourse.tile as tile
from concourse import bacc, bass_utils, masks

N_CORES = 8
N, CIN, H, W = 32, 256, 56, 56
COUT, KS = 256, 3
NPC = N // N_CORES          # images per core
HP, WP = H + 2, W + 2       # padded spatial (58x58)
GRID = HP * WP              # 3364
LEAD = 64                   # per-chunk front pad so tap offsets never go negative
CHUNK = 3440                # LEAD + GRID + 12 tail, %16 == 0 (DoubleRow step)
NROW_GROUPS = 7
ROWS_PER_GROUP = H // NROW_GROUPS   # 8
FREE = ROWS_PER_GROUP * WP          # 464 <= 512 (one PSUM bank, fp32)
CI_CHUNKS = CIN // 128
CO_CHUNKS = COUT // 128

F32 = mybir.dt.float32
FP8 = mybir.dt.float8e4
ALU = mybir.AluOpType
AF = mybir.ActivationFunctionType
DR = mybir.MatmulPerfMode.DoubleRow

# tap groups for the weight-transpose drains: 4+4+1 blocks per 512-f32 PSUM bank
TAP_GROUPS = [(0, 4), (4, 4), (8, 1)]


def _body(tc, x_d, w_d, b_d, o_d, repeats=1):
    nc = tc.nc

    from contextlib import ExitStack
    ctx = ExitStack()
    with ctx:
        const_pool = ctx.enter_context(tc.tile_pool(name="const", bufs=1))
        wd_pool = ctx.enter_context(tc.tile_pool(name="wd", bufs=1))
        wsb_pool = ctx.enter_context(tc.tile_pool(name="wsb", bufs=1))
        xpad_pool = ctx.enter_context(tc.tile_pool(name="xpad", bufs=1))
        xin_pool = ctx.enter_context(tc.tile_pool(name="xin", bufs=3))
        out_pool = ctx.enter_context(tc.tile_pool(name="outs", bufs=2))

        ident = const_pool.tile([128, 128], F32, tag="ident", name="ident")
        masks.make_identity(nc, ident[:])

        bias_sb = const_pool.tile([128, CO_CHUNKS], F32, tag="bias",
                                  name="bias_sb")
        nc.sync.dma_start(bias_sb[:], b_d.rearrange("(c p) -> p c", p=128))

        o_d3 = [[o_d[n, cc * 128:(cc + 1) * 128].rearrange("c h w -> c (h w)")
                 for cc in range(CO_CHUNKS)] for n in range(NPC)]

        for rep in range(repeats):
            # ---- weight phase: one contiguous DMA + on-chip transpose ----
            # wsb: [o_local=128, (oc, i, tap)] — HBM-contiguous (i, tap) on
            # the free axis, so this is 256 descriptors of 9216B.
            wsb = wsb_pool.tile([128, CO_CHUNKS * CIN * KS * KS], F32,
                                tag="wsb", name=f"wsb{rep}")
            nc.sync.dma_start(
                wsb[:].rearrange("p (oc r) -> p oc r", oc=CO_CHUNKS),
                w_d.rearrange("(oc p) i kh kw -> p oc (i kh kw)", p=128))
            wview = wsb[:].rearrange("p (oc i t) -> p oc i t",
                                     oc=CO_CHUNKS, t=KS * KS)

            # wd8[cc]: [128 ci_local, 9*256] fp8, free idx = tap*256 + two*128
            # + co, values (w>=0)-0.5 in {+-0.5}. (lhsT slice per tap:
            # [k][two][m], steps [128, 1] — DoubleRow pairing contracts
            # (k, two) elementwise on both operands.)
            wd8 = []
            for cc in range(CO_CHUNKS):
                wt = wd_pool.tile([128, KS * KS * 256], FP8, tag=f"wd{cc}",
                                  name=f"wd8_{rep}_{cc}")
                wd8.append(wt)
            with tc.tile_pool(name="wtp", bufs=2, space="PSUM") as wtpsum:
                for cc in range(CO_CHUNKS):
                    wt3 = wd8[cc][:].rearrange("k (t x) -> k t x", t=KS * KS)
                    for two in range(CI_CHUNKS):
                        for g, (t0, tn) in enumerate(TAP_GROUPS):
                            pt = wtpsum.tile([128, 512], F32, tag="wtp",
                                             name=f"wtp{rep}_{cc}_{two}_{g}")
                            for j in range(tn):
                                nc.tensor.transpose(
                                    pt[:, j * 128:(j + 1) * 128],
                                    wview[:, cc,
                                          two * 128:(two + 1) * 128, t0 + j],
                                    ident[:])
                            # drain + binarize: {+-0.5} fp8, scattered to
                            # [tap][two][co] (dst strides: tap 256, co 1)
                            nc.vector.tensor_scalar(
                                wt3[:, t0:t0 + tn,
                                    two * 128:(two + 1) * 128],
                                pt[:, :tn * 128].rearrange(
                                    "k (t x) -> k t x", x=128),
                                0.0, 0.5, op0=ALU.is_ge, op1=ALU.subtract)

            # ---- input phase: per-(image, ci-chunk) load + binarize ----
            # one tensor holds all 8 (image, ci-chunk) padded grids; borders
            # zeroed with 6 multi-grid strided memsets (disjoint from the
            # interiors the binarize writes, so they run concurrently)
            xpall = xpad_pool.tile([128, NPC * CI_CHUNKS * CHUNK], FP8,
                                   tag="xpall", name=f"xpall{rep}")
            xg = xpall[:].rearrange("c (g s) -> c g s", s=CHUNK)
            nc.gpsimd.memset(xg[:, :, 0:LEAD], 0.0)
            nc.gpsimd.memset(xg[:, :, LEAD + GRID:CHUNK], 0.0)
            xgrid = xg[:, :, LEAD:LEAD + GRID] \
                .rearrange("c g (h w) -> c g h w", w=WP)
            nc.gpsimd.memset(xgrid[:, :, 0:1, :], 0.0)
            nc.gpsimd.memset(xgrid[:, :, HP - 1:HP, :], 0.0)
            nc.gpsimd.memset(xgrid[:, :, 1:HP - 1, 0:1], 0.0)
            nc.gpsimd.memset(xgrid[:, :, 1:HP - 1, WP - 1:WP], 0.0)
            xg4 = xpall[:].rearrange("c (n t s) -> c n t s",
                                     t=CI_CHUNKS, s=CHUNK)
            for n in range(NPC):
                for two in range(CI_CHUNKS):
                    x_raw = xin_pool.tile([128, H * W], F32, tag="xraw",
                                          name=f"xraw{rep}_{n}_{two}")
                    nc.sync.dma_start(
                        x_raw[:],
                        x_d[n, two * 128:(two + 1) * 128]
                        .rearrange("c h w -> c (h w)"))
                    nc.vector.tensor_scalar(
                        xg4[:, n, two, LEAD:LEAD + GRID]
                        .rearrange("c (h w) -> c h w", w=WP
                                   )[:, 1:H + 1, 1:W + 1],
                        x_raw[:].rearrange("c (h w) -> c h w", w=W),
                        0.0, 0.5, op0=ALU.is_ge, op1=ALU.subtract)
            xp = [xpall[:, n * CI_CHUNKS * CHUNK:(n + 1) * CI_CHUNKS * CHUNK]
                  for n in range(NPC)]

            # ---- conv phase ----
            with tc.tile_pool(name="cpsum", bufs=1, space="PSUM") as cpsum:
                for n in range(NPC):
                    for cc in range(CO_CHUNKS):
                        pp = cpsum.tile([128, NROW_GROUPS * 512], F32,
                                        tag="cps", name=f"cps{rep}_{cc}_{n}",
                                        bufs=1)
                        for kpos in range(KS * KS):
                            kh, kw = divmod(kpos, KS)
                            lhsT = wd8[cc][:, kpos * 256:(kpos + 1) * 256] \
                                .rearrange("k (two m) -> k two m", two=2)
                            for rg in range(NROW_GROUPS):
                                off = (LEAD + WP + rg * FREE
                                       + (kh - 1) * WP + (kw - 1))
                                rhs = xp[n].rearrange(
                                    "k (two s) -> k two s",
                                    s=CHUNK)[:, :, off:off + FREE]
                                nc.tensor.matmul(
                                    pp[:, rg * 512:rg * 512 + FREE], lhsT,
                                    rhs, start=(kpos == 0),
                                    stop=(kpos == KS * KS - 1),
                                    perf_mode=DR)
                        ob = out_pool.tile([128, NROW_GROUPS * ROWS_PER_GROUP * W],
                                           F32, tag="ob",
                                           name=f"ob{rep}_{cc}_{n}")
                        # per-row-group drains (x4 restores the +-0.25
                        # products); next group's matmul into bank rg only
                        # waits for that bank's drain, not the full tile
                        for rg in range(NROW_GROUPS):
                            drain_in = pp[:, rg * 512:rg * 512 + FREE] \
                                .rearrange("m (r c) -> m r c", c=WP
                                           )[:, :, 1:W + 1]
                            drain_out = ob[:].rearrange(
                                "m (g r c) -> m g r c", g=NROW_GROUPS, c=W
                                )[:, rg]
                            nc.scalar.activation(
                                drain_out, drain_in,
                                AF.Identity, bias=bias_sb[:, cc:cc + 1],
                                scale=4.0)
                        nc.sync.dma_start(o_d3[n][cc], ob[:])


_nc_cache = {}


def _get_nc(repeats=1):
    key = repeats
    if key not in _nc_cache:
        nc = bacc.Bacc("TRN2", debug=False)
        x_d = nc.dram_tensor("x", [NPC, CIN, H, W], F32, kind="ExternalInput").ap()
        w_d = nc.dram_tensor("w", [COUT, CIN, KS, KS], F32,
                             kind="ExternalInput").ap()
        b_d = nc.dram_tensor("b", [COUT], F32, kind="ExternalInput").ap()
        o_d = nc.dram_tensor("out", [NPC, COUT, H, W], F32,
                             kind="ExternalOutput").ap()
        with tile.TileContext(nc) as tc:
            _body(tc, x_d, w_d, b_d, o_d, repeats=repeats)
        nc.compile()
        _nc_cache[key] = nc
    return _nc_cache[key]


def _run(inputs, repeats=1, **kwargs):
    x, w, b = inputs["x"], inputs["w"], inputs["b"]
    assert x.shape == (N, CIN, H, W), x.shape
    nc = _get_nc(repeats)
    in_maps = [{
        "x": np.ascontiguousarray(x[i * NPC:(i + 1) * NPC], dtype=np.float32),
        "w": np.ascontiguousarray(w, dtype=np.float32),
        "b": np.ascontiguousarray(b, dtype=np.float32),
    } for i in range(N_CORES)]
    res = bass_utils.run_bass_kernel_spmd(
        nc, in_maps, core_ids=list(range(N_CORES)), **kwargs)
    out = np.concatenate([res.results[i]["out"] for i in range(N_CORES)], axis=0)
    return out, res


def kernel(**inputs) -> np.ndarray:
    out, _ = _run(inputs)
    return out


# revision 9
# speedup vs baseline: 1.0079x; 1.0079x over previous
"""Binarized 3x3 conv (N=32, C=256->256, H=W=56, pad 1) on 8 TRN2 NeuronCores.

Sharding: data-parallel over batch (4 images per core), weights replicated.

Math: binarize exactly via
  xb = (x >= 0) - 0.5            in {+-0.5}  (exact in fp8 e4m3)
  wb = (w >= 0) - 0.5            in {+-0.5}  (exact in fp8 e4m3)
so every product is exactly +-0.25 and fp32 PSUM accumulation is exact
(quarter-integer partial sums, |.| <= 576 << 2^22). The output drain applies
scale=4.0 to restore the +-1-product conv result. sign(0)=+1 is honored.

Conv as matmul: the padded (58x58) binarized image lives flat in SBUF, so for
each kernel tap (kh,kw) the needed input window is a CONTIGUOUS span of the
flat padded grid shifted by (kh-1)*58+(kw-1). Outputs are computed on the
padded grid (464-wide spans = 8 padded rows) and the two garbage columns per
row (conv centered on pad columns) are dropped at drain time.

TensorE: fp8 DoubleRow matmuls contract all 256 input channels in one
instruction (K=128 partitions x 2 interleaved weights/cell), 9 accumulating
matmuls (one per tap) per output tile. 2 co-chunks x 4 images x 7 row-groups
x 9 taps = 504 matmuls per core.

Weights: ONE contiguous DMA loads w[o, i, kh, kw] as [o_local=128 part,
(oc, i, tap)] (256 descriptors of 9216B — the HBM-contiguous axis (i, tap)
lands on the SBUF free axis). The o<->i transpose needed for the matmul
lhsT layout [ci_local][two][co] is done on-chip: 36 PE transpose-mode
matmuls of 128x128 f32 blocks (strided columns, stride 9) into PSUM, each
drained by a DVE tensor_scalar that fuses the binarize to {+-0.5} fp8 and
scatters into the DoubleRow layout [tap][two][co]. This replaces the old
36B-run gather DMA (131072 descriptors, ~38 ms) with ~25 us of work.
"""

import os
os.environ.setdefault("CONCOURSE_SCRUB_NEFF_DEBUG_INFO", "1")

import numpy as np

import concourse.bass as bass
import concourse.mybir as mybir
import concourse.tile as tile
from concourse import bacc, bass_utils, masks

N_CORES = 8
N, CIN, H, W = 32, 256, 56, 56
COUT, KS = 256, 3
NPC = N // N_CORES          # images per core
HP, WP = H + 2, W + 2       # padded spatial (58x58)
GRID = HP * WP              # 3364
LEAD = 64                   # per-chunk front pad so tap offsets never go negative
CHUNK = 3440                # LEAD + GRID + 12 tail, %16 == 0 (DoubleRow step)
NROW_GROUPS = 7
ROWS_PER_GROUP = H // NROW_GROUPS   # 8
FREE = ROWS_PER_GROUP * WP          # 464 <= 512 (one PSUM bank, fp32)
CI_CHUNKS = CIN // 128
CO_CHUNKS = COUT // 128

F32 = mybir.dt.float32
FP8 = mybir.dt.float8e4
ALU = mybir.AluOpType
AF = mybir.ActivationFunctionType
DR = mybir.MatmulPerfMode.DoubleRow

# tap groups for the weight-transpose drains: 4+4+1 blocks per 512-f32 PSUM bank
TAP_GROUPS = [(0, 4), (4, 4), (8, 1)]


def _body(tc, x_d, w_d, b_d, o_d, repeats=1):
    nc = tc.nc

    from contextlib import ExitStack
    ctx = ExitStack()
    with ctx:
        const_pool = ctx.enter_context(tc.tile_pool(name="const", bufs=1))
        wd_pool = ctx.enter_context(tc.tile_pool(name="wd", bufs=1))
        wsb_pool = ctx.enter_context(tc.tile_pool(name="wsb", bufs=1))
        xpad_pool = ctx.enter_context(tc.tile_pool(name="xpad", bufs=1))
        xin_pool = ctx.enter_context(tc.tile_pool(name="xin", bufs=3))
        out_pool = ctx.enter_context(tc.tile_pool(name="outs", bufs=2))

        ident = const_pool.tile([128, 128], F32, tag="ident", name="ident")
        masks.make_identity(nc, ident[:])

        bias_sb = const_pool.tile([128, CO_CHUNKS], F32, tag="bias",
                                  name="bias_sb")

        o_d3 = [[o_d[n, cc * 128:(cc + 1) * 128].rearrange("c h w -> c (h w)")
                 for cc in range(CO_CHUNKS)] for n in range(NPC)]

        for rep in range(repeats):
            # ---- weight phase: contiguous DMAs + on-chip transpose ----
            # wsb: [o_local=128, (oc, i, tap)] — HBM-contiguous (i, tap) on
            # the free axis, so this is 128 descriptors of 9216B per chunk.
            # Split per co-chunk and issued on the ACT HWDGE ring (nc.scalar)
            # so cc0's transposes start at ~3.5us while the SP ring streams x.
            wsb = wsb_pool.tile([128, CO_CHUNKS * CIN * KS * KS], F32,
                                tag="wsb", name=f"wsb{rep}")
            wsb_oc = wsb[:].rearrange("p (oc r) -> p oc r", oc=CO_CHUNKS)
            w_src = w_d.rearrange("(oc p) i kh kw -> p oc (i kh kw)", p=128)
            for cc in range(CO_CHUNKS):
                nc.scalar.dma_start(wsb_oc[:, cc], w_src[:, cc])
            if rep == 0:
                nc.scalar.dma_start(bias_sb[:],
                                    b_d.rearrange("(c p) -> p c", p=128))
            wview = wsb[:].rearrange("p (oc i t) -> p oc i t",
                                     oc=CO_CHUNKS, t=KS * KS)

            # wd8[cc]: [128 ci_local, 9*256] fp8, free idx = tap*256 + two*128
            # + co, values (w>=0)-0.5 in {+-0.5}. (lhsT slice per tap:
            # [k][two][m], steps [128, 1] — DoubleRow pairing contracts
            # (k, two) elementwise on both operands.)
            wd8 = []
            for cc in range(CO_CHUNKS):
                wt = wd_pool.tile([128, KS * KS * 256], FP8, tag=f"wd{cc}",
                                  name=f"wd8_{rep}_{cc}")
                wd8.append(wt)
            with tc.tile_pool(name="wtp", bufs=2, space="PSUM") as wtpsum:
                for cc in range(CO_CHUNKS):
                    wt3 = wd8[cc][:].rearrange("k (t x) -> k t x", t=KS * KS)
                    for two in range(CI_CHUNKS):
                        for g, (t0, tn) in enumerate(TAP_GROUPS):
                            pt = wtpsum.tile([128, 512], F32, tag="wtp",
                                             name=f"wtp{rep}_{cc}_{two}_{g}")
                            for j in range(tn):
                                nc.tensor.transpose(
                                    pt[:, j * 128:(j + 1) * 128],
                                    wview[:, cc,
                                          two * 128:(two + 1) * 128, t0 + j],
                                    ident[:])
                            # drain + binarize: {+-0.5} fp8, scattered to
                            # [tap][two][co] (dst strides: tap 256, co 1)
                            nc.vector.tensor_scalar(
                                wt3[:, t0:t0 + tn,
                                    two * 128:(two + 1) * 128],
                                pt[:, :tn * 128].rearrange(
                                    "k (t x) -> k t x", x=128),
                                0.0, 0.5, op0=ALU.is_ge, op1=ALU.subtract)

            # ---- input phase: per-(image, ci-chunk) load + binarize ----
            # one tensor holds all 8 (image, ci-chunk) padded grids; borders
            # zeroed with 6 multi-grid strided memsets (disjoint from the
            # interiors the binarize writes, so they run concurrently)
            xpall = xpad_pool.tile([128, NPC * CI_CHUNKS * CHUNK], FP8,
                                   tag="xpall", name=f"xpall{rep}")
            xg = xpall[:].rearrange("c (g s) -> c g s", s=CHUNK)
            nc.gpsimd.memset(xg[:, :, 0:LEAD], 0.0)
            nc.gpsimd.memset(xg[:, :, LEAD + GRID:CHUNK], 0.0)
            xgrid = xg[:, :, LEAD:LEAD + GRID] \
                .rearrange("c g (h w) -> c g h w", w=WP)
            nc.gpsimd.memset(xgrid[:, :, 0:1, :], 0.0)
            nc.gpsimd.memset(xgrid[:, :, HP - 1:HP, :], 0.0)
            nc.gpsimd.memset(xgrid[:, :, 1:HP - 1, 0:1], 0.0)
            nc.gpsimd.memset(xgrid[:, :, 1:HP - 1, WP - 1:WP], 0.0)
            xg4 = xpall[:].rearrange("c (n t s) -> c n t s",
                                     t=CI_CHUNKS, s=CHUNK)
            for n in range(NPC):
                for two in range(CI_CHUNKS):
                    x_raw = xin_pool.tile([128, H * W], F32, tag="xraw",
                                          name=f"xraw{rep}_{n}_{two}")
                    nc.sync.dma_start(
                        x_raw[:],
                        x_d[n, two * 128:(two + 1) * 128]
                        .rearrange("c h w -> c (h w)"))
                    nc.vector.tensor_scalar(
                        xg4[:, n, two, LEAD:LEAD + GRID]
                        .rearrange("c (h w) -> c h w", w=WP
                                   )[:, 1:H + 1, 1:W + 1],
                        x_raw[:].rearrange("c (h w) -> c h w", w=W),
                        0.0, 0.5, op0=ALU.is_ge, op1=ALU.subtract)
            xp = [xpall[:, n * CI_CHUNKS * CHUNK:(n + 1) * CI_CHUNKS * CHUNK]
                  for n in range(NPC)]

            # ---- conv phase ----
            # per-row-group PSUM tiles rotating through all 8 banks: group
            # g+1's first matmul into a bank only waits for a drain from
            # ~1.5 groups earlier, so TensorE never stalls on drains
            with tc.tile_pool(name="cpsum", bufs=8, space="PSUM") as cpsum:
                ngroups = NPC * CO_CHUNKS
                for gi in range(ngroups):
                    n, cc = divmod(gi, CO_CHUNKS)
                    pps = [cpsum.tile([128, 512], F32, tag="cps",
                                      name=f"cps{rep}_{cc}_{n}_{rg}")
                           for rg in range(NROW_GROUPS)]
                    for kpos in range(KS * KS):
                        kh, kw = divmod(kpos, KS)
                        lhsT = wd8[cc][:, kpos * 256:(kpos + 1) * 256] \
                            .rearrange("k (two m) -> k two m", two=2)
                        for rg in range(NROW_GROUPS):
                            off = (LEAD + WP + rg * FREE
                                   + (kh - 1) * WP + (kw - 1))
                            rhs = xp[n].rearrange(
                                "k (two s) -> k two s",
                                s=CHUNK)[:, :, off:off + FREE]
                            nc.tensor.matmul(
                                pps[rg][:, :FREE], lhsT,
                                rhs, start=(kpos == 0),
                                stop=(kpos == KS * KS - 1),
                                perf_mode=DR)
                    ob = out_pool.tile([128, NROW_GROUPS * ROWS_PER_GROUP * W],
                                       F32, tag="ob",
                                       name=f"ob{rep}_{cc}_{n}")
                    # per-row-group drains (x4 restores the +-0.25 products),
                    # alternating ACT/DVE so the serial drain tail halves
                    for rg in range(NROW_GROUPS):
                        drain_in = pps[rg][:, :FREE] \
                            .rearrange("m (r c) -> m r c", c=WP
                                       )[:, :, 1:W + 1]
                        drain_out = ob[:].rearrange(
                            "m (g r c) -> m g r c", g=NROW_GROUPS, c=W
                            )[:, rg]
                        if rg % 2 == 0:
                            nc.scalar.activation(
                                drain_out, drain_in,
                                AF.Identity, bias=bias_sb[:, cc:cc + 1],
                                scale=4.0)
                        else:
                            nc.vector.tensor_scalar(
                                drain_out, drain_in,
                                4.0, bias_sb[:, cc:cc + 1],
                                op0=ALU.mult, op1=ALU.add)
                    # outputs ride the idle SWDGE/Pool path so they never
                    # contend with the SP ring streaming x; the last group
                    # is split so its early quarters overlap the final drains
                    ob_g = ob[:].rearrange("m (g s) -> m g s", g=NROW_GROUPS)
                    od_g = o_d3[n][cc].rearrange("c (g s) -> c g s",
                                                 g=NROW_GROUPS)
                    if gi == ngroups - 1:
                        for lo, hi in ((0, 2), (2, 4), (4, 6), (6, 7)):
                            nc.gpsimd.dma_start(od_g[:, lo:hi],
                                                ob_g[:, lo:hi])
                    else:
                        nc.gpsimd.dma_start(o_d3[n][cc], ob[:])


_nc_cache = {}


def _get_nc(repeats=1):
    key = repeats
    if key not in _nc_cache:
        nc = bacc.Bacc("TRN2", debug=False)
        x_d = nc.dram_tensor("x", [NPC, CIN, H, W], F32, kind="ExternalInput").ap()
        w_d = nc.dram_tensor("w", [COUT, CIN, KS, KS], F32,
                             kind="ExternalInput").ap()
        b_d = nc.dram_tensor("b", [COUT], F32, kind="ExternalInput").ap()
        o_d = nc.dram_tensor("out", [NPC, COUT, H, W], F32,
                             kind="ExternalOutput").ap()
        with tile.TileContext(nc) as tc:
            _body(tc, x_d, w_d, b_d, o_d, repeats=repeats)
        nc.compile()
        _nc_cache[key] = nc
    return _nc_cache[key]


def _run(inputs, repeats=1, **kwargs):
    x, w, b = inputs["x"], inputs["w"], inputs["b"]
    assert x.shape == (N, CIN, H, W), x.shape
    nc = _get_nc(repeats)
    in_maps = [{
        "x": np.ascontiguousarray(x[i * NPC:(i + 1) * NPC], dtype=np.float32),
        "w": np.ascontiguousarray(w, dtype=np.float32),
        "b": np.ascontiguousarray(b, dtype=np.float32),
    } for i in range(N_CORES)]
    res = bass_utils.run_bass_kernel_spmd(
        nc, in_maps, core_ids=list(range(N_CORES)), **kwargs)
    out = np.concatenate([res.results[i]["out"] for i in range(N_CORES)], axis=0)
    return out, res


def kernel(**inputs) -> np.ndarray:
    out, _ = _run(inputs)
    return out
